# revision 34
# baseline (speedup 1.0000x reference)
"""Trainium2 Bass kernel for nn_AtLocPlusCriterion_VO.

loss = exp(-srx)*mean|vo_t - tg_t| + srx + exp(-srq)*mean|vo_q - tg_q| + srq
with vo = calc_vo_logq(pred[:-1], pred[1:]) (relative SE(3) pose, log-quaternion).

Sequence-parallel across 8 NeuronCores (1-row halo per shard). Inputs are
resharded host-side into component-major (SoA) bf16 planes so every on-device
vector op runs in the DVE 2x bf16 mode. Per core: 1956 pairs per SBUF
partition, 3 tiles of 652.

Row phase: n^2 on VectorE, 1/n and n/2 via Ln/Exp, half-angle sin/cos via the
Sin LUT (A = sqrt2*cos n from 1-2sin^2(n/2), U = sqrt2*sin(n)/n * v). Pair
phase on VectorE: rotation by two cross products, quaternion product, log map
via arctan. Cross-product / quaternion-product partial products are emitted
as merged 6C-wide DVE ops (4D access patterns pairing two 3-vector products
per instruction). Unary work (squares, LUTs, |x| + accumulate via accum_out)
runs on ScalarE; activation-table switches are grouped (row Ln/Exp+Sin of
tile 0 first so VectorE starts early, then tiles 1-2, then per-tile pair
Ln/Exp+Arctan). The translation-delta g1 for tiles 0-1 is computed early to
fill VectorE's wait on the scalar LUT prologue. Final mean-L1 partial sums
(6 x [128,1] f32) DMA straight to HBM; the host does the last reduction.
"""
import os
import numpy as np
import ml_dtypes

N_CORES = 8
T_FULL = 2_000_000
NPAIRS = T_FULL - 1          # 1_999_999
D = 1956                     # pairs per partition per core
C = 652                      # pairs per tile (3 tiles)
NT = 3
R = C + 1
PPC = 128 * D                # 250_368 pairs per core
PAIRS_PAD = N_CORES * PPC    # 2_002_944
ROWS_PAD = PAIRS_PAD + 1

PRED_LEN = 6 * (PPC + 1)
TARG_LEN = 6 * PPC

LN2 = float(np.log(2.0))
LN2SQ2 = float(np.log(2.0 * np.sqrt(2.0)))   # i2n carries 2*sqrt2
PI2 = float(np.pi / 2.0)
SQ2 = float(np.sqrt(2.0))

_BUILT = {}


def _patch_act_tables():
    import concourse.bacc as bacc_mod
    import concourse.hw_specs as hw

    if getattr(bacc_mod, "_vo_tables_patched", False):
        return
    orig = hw.get_activation_tables

    def steered(arch, _orig=orig):
        from concourse import mybir as _mb
        AF = _mb.ActivationFunctionType
        t = {k: set(v) for k, v in _orig(arch).items()}
        # Keep all 24 entries (act_func_set_id indexes the original list);
        # drop ln/exp/arctan from the earlier sets so the table-load pass
        # resolves them to natural_log_exp_and_others / trig_and_small.
        t.get("natural_log", set()).discard(AF.Ln)
        t.get("exp_and_others", set()).discard(AF.Exp)
        t.get("sigmoid_and_others", set()).discard(AF.Arctan)
        return t

    bacc_mod.get_activation_tables = steered
    bacc_mod._vo_tables_patched = True


def _build():
    from concourse import bacc, tile, mybir
    from concourse.ap import AP
    from concourse.bass import _add_dep_helper

    _patch_act_tables()

    f32, bf16 = mybir.dt.float32, mybir.dt.bfloat16
    OP = mybir.AluOpType
    AF = mybir.ActivationFunctionType

    nc = bacc.Bacc("TRN2", target_bir_lowering=False, debug=False,
                   num_devices=N_CORES)
    pred_h = nc.declare_dram_parameter("pred", [PRED_LEN], bf16, isOutput=False)
    targ_h = nc.declare_dram_parameter("targ", [TARG_LEN], bf16, isOutput=False)
    out_h = nc.declare_dram_parameter("out", [128, 6], f32, isOutput=True)

    for v in (1e-16, -LN2, LN2SQ2, PI2):
        v = float(v)
        if (f32, v) not in nc.const_aps.aps:
            t = nc.alloc_sbuf_tensor(f"uconst-{v}", [128, 1], f32)
            nc.gpsimd.memset(t.ap(), v)
            nc.const_aps.aps[(f32, v)] = t.ap()
    nc.all_engine_barrier()

    PL_P = PPC + 1   # pred plane length
    PL_T = PPC       # targ plane length

    def sb(tile_, off, dims):
        base = tile_[:, :]
        return AP(base.tensor, base.offset + off,
                  [[base.ap.to_list()[0][0], 128]] + dims)

    acc_ts, acc_qs = [], []
    groups = {}  # (tile, name) -> list of act instructions

    with tile.TileContext(nc) as tc:

        def mkact(tile_i, group, *args, **kw):
            ins = nc.scalar.activation(*args, **kw)
            if group is not None:
                groups.setdefault((tile_i, group), []).append(ins)
            return ins

        with (
            tc.tile_pool(name="inp", bufs=1) as pin,
            tc.tile_pool(name="rowp", bufs=1) as prow,
            tc.tile_pool(name="scr", bufs=1) as pscr,
            tc.tile_pool(name="accp", bufs=8) as pacc,
        ):
            state = {}

            def dma_phase(t):
                # ---- DMA: component-major bf16 planes, all contiguous ----
                tv = pin.tile([128, 3 * R], bf16, tag=f"tv{t}")   # logq comps
                nc.sync.dma_start(
                    tv[:].rearrange("p (c r) -> p c r", c=3),
                    AP(pred_h, 3 * PL_P + t * C, [[D, 128], [PL_P, 3], [1, R]]))
                tt = pin.tile([128, 3 * R], bf16, tag=f"tt{t}")   # t comps
                nc.sync.dma_start(
                    tt[:].rearrange("p (c r) -> p c r", c=3),
                    AP(pred_h, t * C, [[D, 128], [PL_P, 3], [1, R]]))
                gtt = pin.tile([128, 3 * C], bf16, tag=f"gtt{t}")  # targ t
                nc.sync.dma_start(
                    gtt[:].rearrange("p (c r) -> p c r", c=3),
                    AP(targ_h, t * C, [[D, 128], [PL_T, 3], [1, C]]))
                gtq = pin.tile([128, 3 * C], bf16, tag=f"gtq{t}")  # targ q
                nc.sync.dma_start(
                    gtq[:].rearrange("p (c r) -> p c r", c=3),
                    AP(targ_h, 3 * PL_T + t * C, [[D, 128], [PL_T, 3], [1, C]]))
                state[t] = {"tv": tv, "tt": tt, "gtt": gtt, "gtq": gtq}

            def row_n2(t):
                st = state[t]
                sq = pscr.tile([128, 3 * R], bf16, tag="sq")
                nc.vector.tensor_tensor(sq[:], st["tv"][:], st["tv"][:],
                                        OP.mult)
                n2a = pscr.tile([128, R], bf16, tag="n2a")
                nc.vector.tensor_tensor(n2a[:], sq[:, 0:R], sq[:, R:2 * R], OP.add)
                n2 = pscr.tile([128, R], bf16, tag=f"n2{t}")
                nc.vector.tensor_tensor(n2[:], n2a[:], sq[:, 2 * R:3 * R], OP.add)
                st["n2"] = n2

            def early_g1(t):
                # translation deltas: independent of the row LUT chain, runs
                # on VectorE while ScalarE does table loads + Ln/Exp/Sin.
                st = state[t]
                tt = st["tt"]
                g1 = pscr.tile([128, 5 * C], bf16, tag=f"g1{t}")
                nc.vector.tensor_tensor(
                    sb(g1, 0, [[C, 3], [1, C]]),
                    sb(tt, 1, [[R, 3], [1, C]]),
                    sb(tt, 0, [[R, 3], [1, C]]), OP.subtract)
                nc.vector.tensor_copy(g1[:, 3 * C:5 * C], g1[:, 0:2 * C])
                st["g1"] = g1

            def row_lut(t):
                st = state[t]
                l = pscr.tile([128, R], f32, tag="l")
                mkact(t, 'rowLE', l[:], st["n2"][:], AF.Ln, bias=1e-16)
                n4 = pscr.tile([128, R], f32, tag=f"n4{t}")
                mkact(t, 'rowLE', n4[:], l[:], AF.Exp, bias=-LN2, scale=0.5)
                i2n = pscr.tile([128, R], bf16, tag=f"i2n{t}")
                mkact(t, 'rowLE', i2n[:], l[:], AF.Exp, bias=LN2SQ2, scale=-0.5)
                st["n4"], st["i2n"] = n4, i2n

            def row_trig(t):
                st = state[t]
                s4 = pscr.tile([128, R], bf16, tag=f"s4{t}")
                mkact(t, 'rowTR', s4[:], st["n4"][:], AF.Sin)
                c4 = pscr.tile([128, R], bf16, tag=f"c4{t}")
                mkact(t, 'rowTR', c4[:], st["n4"][:], AF.Sin, bias=PI2)
                st["s4"], st["c4"] = s4, c4

            def row_fin(t):
                st = state[t]
                sc = pscr.tile([128, R], bf16, tag="scx")
                nc.vector.tensor_tensor(sc[:], st["s4"][:], st["c4"][:], OP.mult)
                s4sq = pscr.tile([128, R], bf16, tag="s4sq")
                mkact(t, None, s4sq[:], st["s4"][:], AF.Square)
                A = prow.tile([128, R], bf16, tag=f"A{t}")
                mkact(t, None, A[:], s4sq[:], AF.Copy, bias=SQ2, scale=-2.0 * SQ2)
                sn = pscr.tile([128, R], bf16, tag="sn")
                nc.vector.tensor_tensor(sn[:], sc[:], st["i2n"][:], OP.mult)
                U = prow.tile([128, 5 * R], bf16, tag=f"U{t}")
                nc.vector.tensor_tensor(
                    sb(U, 0, [[R, 3], [1, R]]),
                    st["tv"][:].rearrange("p (c r) -> p c r", c=3),
                    sb(sn, 0, [[0, 3], [1, R]]), OP.mult)
                nc.vector.tensor_copy(U[:, 3 * R:5 * R], U[:, 0:2 * R])
                st["A"], st["U"] = A, U

            def pair_helpers(t):
                st = state[t]
                A, U, tt = st["A"], st["U"], st["tt"]

                def A_at(row_off):
                    return sb(A, row_off, [[0, 3], [1, C]])

                def U_at(comp_rot, row_off):
                    return sb(U, comp_rot * R + row_off, [[R, 3], [1, C]])

                def TT_at(row_off):
                    return sb(tt, row_off, [[R, 3], [1, C]])

                cmC = lambda tl: sb(tl, 0, [[C, 3], [1, C]])
                return st, A, U, A_at, U_at, TT_at, cmC

            def pair_t_branch(t):
                st, A, U, A_at, U_at, TT_at, cmC = pair_helpers(t)
                # ----- translation part -----
                if "g1" in st:
                    g1 = st["g1"]
                else:
                    g1 = pscr.tile([128, 5 * C], bf16, tag="g10")
                    nc.vector.tensor_tensor(cmC(g1), TT_at(1), TT_at(0),
                                            OP.subtract)
                    nc.vector.tensor_copy(g1[:, 3 * C:5 * C], g1[:, 0:2 * C])

                def G1(comp_rot):
                    return sb(g1, comp_rot * C, [[C, 3], [1, C]])

                pp = pscr.tile([128, 6 * C], bf16, tag="pp")
                nc.vector.tensor_tensor(
                    sb(pp, 0, [[3 * C, 2], [C, 3], [1, C]]),
                    sb(U, R, [[R, 2], [R, 3], [1, C]]),
                    sb(g1, 2 * C, [[-C, 2], [C, 3], [1, C]]), OP.mult)
                b = pscr.tile([128, 5 * C], bf16, tag="b")
                nc.vector.tensor_tensor(cmC(b), pp[:, 0:3 * C],
                                        pp[:, 3 * C:6 * C], OP.subtract)
                nc.vector.tensor_copy(b[:, 3 * C:5 * C], b[:, 0:2 * C])

                def B(comp_rot):
                    return sb(b, comp_rot * C, [[C, 3], [1, C]])

                qq = pscr.tile([128, 6 * C], bf16, tag="pp")
                nc.vector.tensor_tensor(
                    sb(qq, 0, [[3 * C, 2], [C, 3], [1, C]]),
                    sb(U, R, [[R, 2], [R, 3], [1, C]]),
                    sb(b, 2 * C, [[-C, 2], [C, 3], [1, C]]), OP.mult)
                cp = pscr.tile([128, 3 * C], bf16, tag="cp")
                nc.vector.tensor_tensor(cmC(cp), qq[:, 0:3 * C],
                                        qq[:, 3 * C:6 * C], OP.subtract)
                m = pscr.tile([128, 3 * C], bf16, tag="m")
                nc.vector.tensor_tensor(cmC(m), A_at(0), B(0), OP.mult)

                g = pscr.tile([128, 3 * C], bf16, tag="gg")
                nc.vector.tensor_tensor(g[:], g1[:, 0:3 * C], st["gtt"][:],
                                        OP.subtract)
                gc = pscr.tile([128, 3 * C], bf16, tag="gc")
                nc.vector.tensor_tensor(gc[:], g[:], cp[:], OP.add)
                dfft = pscr.tile([128, 3 * C], bf16, tag="dfft")
                nc.vector.tensor_tensor(dfft[:], gc[:], m[:], OP.subtract)
                st["dfft"] = dfft

            def pair_abs_t(t):
                st = state[t]
                dump_t = pscr.tile([128, 3 * R], bf16, tag="sq")
                acc_t = pacc.tile([128, 1], f32, tag="acct")
                mkact(t, None, dump_t[:, 0:3 * C], st["dfft"][:], AF.Abs,
                      accum_out=acc_t[:])
                acc_ts.append(acc_t)

            def pair_q_branch(t, sq_on_v=False, abs_t_after_sq=False):
                st, A, U, A_at, U_at, TT_at, cmC = pair_helpers(t)
                # ----- rotation part: qV = A0*U1 - A1*U0 - U0 x U1 (= 2*qv) --
                mc = pscr.tile([128, 7 * C], bf16, tag="pp")
                nc.vector.tensor_tensor(
                    sb(mc, 0, [[3 * C, 2], [C, 3], [1, C]]),
                    sb(U, 0, [[R, 2], [R, 3], [1, C]]),
                    sb(U, 1, [[2 * R, 2], [R, 3], [1, C]]), OP.mult)
                nc.vector.tensor_tensor(mc[:, 6 * C:7 * C], A[:, 0:C],
                                        A[:, 1:1 + C], OP.mult)
                uv = pscr.tile([128, 2 * C], bf16, tag="s1")
                nc.vector.tensor_tensor(
                    sb(uv, 0, [[C, 2], [1, C]]),
                    sb(mc, 0, [[2 * C, 2], [1, C]]),
                    sb(mc, C, [[5 * C, 2], [1, C]]), OP.add)
                qs2 = pscr.tile([128, C], bf16, tag="qs2")
                nc.vector.tensor_tensor(qs2[:], uv[:, 0:C], uv[:, C:2 * C],
                                        OP.add)

                c2 = pscr.tile([128, 3 * C], bf16, tag="p2")
                nc.vector.tensor_tensor(cmC(c2), U_at(2, 0), U_at(1, 1), OP.mult)
                cr = pscr.tile([128, 3 * C], bf16, tag="cr")
                nc.vector.tensor_tensor(cr[:], mc[:, 3 * C:6 * C], c2[:],
                                        OP.subtract)
                pq = pscr.tile([128, 6 * C], bf16, tag="pp")
                nc.vector.tensor_tensor(
                    sb(pq, 0, [[3 * C, 2], [C, 3], [1, C]]),
                    sb(A, 0, [[1, 2], [0, 3], [1, C]]),
                    sb(U, 1, [[-1, 2], [R, 3], [1, C]]), OP.mult)
                w1 = pscr.tile([128, 3 * C], bf16, tag="w1")
                nc.vector.tensor_tensor(w1[:], pq[:, 0:3 * C],
                                        pq[:, 3 * C:6 * C], OP.subtract)
                qV = pscr.tile([128, 3 * C], bf16, tag=f"qV{t}")
                nc.vector.tensor_tensor(qV[:], w1[:], cr[:], OP.subtract)

                qVsq = pscr.tile([128, 3 * C], bf16, tag="p1")
                if sq_on_v:
                    nc.vector.tensor_tensor(qVsq[:], qV[:], qV[:], OP.mult)
                else:
                    mkact(t, None, qVsq[:], qV[:], AF.Square)
                if abs_t_after_sq:
                    pair_abs_t(t)
                nva = pscr.tile([128, C], bf16, tag="s1")
                nc.vector.tensor_tensor(nva[:], qVsq[:, 0:C], qVsq[:, C:2 * C],
                                        OP.add)
                nv2 = pscr.tile([128, C], bf16, tag="s2")
                nc.vector.tensor_tensor(nv2[:], nva[:], qVsq[:, 2 * C:3 * C],
                                        OP.add)

                lq = pscr.tile([128, C], f32, tag="lq")
                mkact(t, 'pairLE', lq[:], nv2[:], AF.Ln, bias=1e-16)
                rs = pscr.tile([128, C], bf16, tag=f"rs{t}")
                mkact(t, 'pairLE', rs[:], lq[:], AF.Exp, scale=-0.5)
                r2 = pscr.tile([128, C], bf16, tag=f"r2{t}")
                nc.vector.tensor_tensor(r2[:], qs2[:], rs[:], OP.mult)
                st["qV"], st["rs"], st["r2"] = qV, rs, r2

            def pair_at(t):
                st = state[t]
                at = pscr.tile([128, C], f32, tag="at")
                mkact(t, 'pairTR', at[:], st["r2"][:], AF.Arctan, scale=-1.0)
                st["at"] = at
                if t < 2:
                    atp = pscr.tile([128, C], bf16, tag="atp")
                    mkact(t, None, atp[:], at[:], AF.Copy, bias=PI2)
                    st["atp"] = atp

            def pair_b_rest(t):
                st = state.pop(t)
                ratio = pscr.tile([128, C], bf16, tag="ratio")
                if "atp" in st:
                    nc.vector.tensor_tensor(ratio[:], st["atp"][:], st["rs"][:],
                                            OP.mult)
                else:
                    nc.vector.scalar_tensor_tensor(ratio[:], st["at"][:], PI2,
                                                   st["rs"][:], OP.add, OP.mult)
                ld = pscr.tile([128, 3 * C], bf16, tag="w1")
                nc.vector.tensor_tensor(
                    sb(ld, 0, [[C, 3], [1, C]]),
                    sb(st["qV"], 0, [[C, 3], [1, C]]),
                    sb(ratio, 0, [[0, 3], [1, C]]), OP.mult)
                ldiff = pscr.tile([128, 3 * C], bf16, tag="cr")
                nc.vector.tensor_tensor(ldiff[:], ld[:], st["gtq"][:], OP.subtract)
                acc_q = pacc.tile([128, 1], f32, tag="accq")
                if t == 2:
                    dump_q = pscr.tile([128, 3 * C], bf16, tag="dfft")
                    nc.vector.scalar_tensor_tensor(dump_q[:], ldiff[:], -1.0,
                                                   ldiff[:], OP.mult, OP.max,
                                                   accum_out=acc_q[:])
                else:
                    dump_q = pscr.tile([128, 3 * R], bf16, tag="sq")
                    mkact(t, None, dump_q[:, 0:3 * C], ldiff[:], AF.Abs,
                          accum_out=acc_q[:])
                acc_qs.append(acc_q)

            for t in range(NT):
                dma_phase(t)
            row_n2(0)
            row_lut(0)
            row_trig(0)
            row_n2(1)
            row_n2(2)
            early_g1(0)
            early_g1(1)
            row_lut(1)
            row_lut(2)
            row_trig(1)
            row_trig(2)
            for t in range(NT):
                row_fin(t)
            for t in range(NT):
                pair_t_branch(t)
                pair_q_branch(t, abs_t_after_sq=True, sq_on_v=(t == 2))
                pair_at(t)
                pair_b_rest(t)

            # Chain LUT activations so same-table-set groups run contiguously
            # across tiles: 4 table loads total (rowLE, rowTR, pairLE, pairTR).
            order = [('rowLE', 0), ('rowTR', 0),
                     ('rowLE', 1), ('rowLE', 2),
                     ('rowTR', 1), ('rowTR', 2)]
            for ti in range(NT):
                order.append(('pairLE', ti))
                order.append(('pairTR', ti))
            seq = []
            for gname, ti in order:
                seq.extend(groups.get((ti, gname), []))
            for i in range(1, len(seq)):
                _add_dep_helper(seq[i].ins, seq[i - 1].ins, False,
                                "act table-set grouping")

            for i, acc in enumerate(acc_ts + acc_qs):
                nc.sync.dma_start(out_h[:, i:i + 1], acc[:])

    nc.compile()
    return nc


def _get_nc():
    if "nc" not in _BUILT:
        _BUILT["nc"] = _build()
    return _BUILT["nc"]


def run_device(pred, targ, trace=False):
    """pred: (1,T,6) f32, targ: (1,T-1,6) f32 -> (sum|dt|, sum|dq|, exec_ns)"""
    from concourse.bass_utils import run_bass_kernel_spmd

    nc = _get_nc()
    p = np.asarray(pred, dtype=np.float32).reshape(-1, 6)
    g = np.asarray(targ, dtype=np.float32).reshape(-1, 6)
    n_dup = ROWS_PAD - p.shape[0]
    p_pad = np.concatenate([p, np.repeat(p[-1:], n_dup, axis=0)], axis=0)
    g_pad = np.concatenate(
        [g, np.zeros((PAIRS_PAD - g.shape[0], 6), np.float32)], axis=0)

    in_maps = []
    for c in range(N_CORES):
        s = c * PPC
        in_maps.append({
            "pred": np.ascontiguousarray(p_pad[s:s + PPC + 1].T)
                     .astype(ml_dtypes.bfloat16).reshape(-1),
            "targ": np.ascontiguousarray(g_pad[s:s + PPC].T)
                     .astype(ml_dtypes.bfloat16).reshape(-1),
        })
    res = run_bass_kernel_spmd(nc, in_maps, core_ids=list(range(N_CORES)),
                               trace=trace)
    psum = np.stack([res.results[i]["out"] for i in range(N_CORES)])
    st = float(psum[:, :, 0:3].sum(dtype=np.float64))
    sq = float(psum[:, :, 3:6].sum(dtype=np.float64))
    return st, sq, res.exec_time_ns


def kernel(pred, targ, srx, srq):
    trace = bool(int(os.environ.get("VO_KERNEL_TRACE", "0")))
    st, sq, _ = run_device(pred, targ, trace=trace)
    t_loss = st / (3.0 * NPAIRS)
    q_loss = sq / (3.0 * NPAIRS)
    srx_v = float(np.asarray(srx).reshape(-1)[0])
    srq_v = float(np.asarray(srq).reshape(-1)[0])
    out = (np.exp(-srx_v) * t_loss + srx_v +
           np.exp(-srq_v) * q_loss + srq_v)
    return np.array([out], dtype=np.float32)


# revision 35
# speedup vs baseline: 1.1660x; 1.1660x over previous
"""Trainium2 Bass kernel for nn_AtLocPlusCriterion_VO.

loss = exp(-srx)*mean|vo_t - tg_t| + srx + exp(-srq)*mean|vo_q - tg_q| + srq
with vo = calc_vo_logq(pred[:-1], pred[1:]) (relative SE(3) pose, log-quaternion).

Sequence-parallel across 8 NeuronCores (1-row halo per shard). Inputs are
resharded host-side into component-major (SoA) bf16 planes so every on-device
vector op runs in the DVE 2x bf16 mode. Per core: 1956 pairs per SBUF
partition, 3 tiles of 652.

Row phase: n^2 on VectorE, 1/n and n/2 via Ln/Exp, half-angle sin/cos via the
Sin LUT (A = sqrt2*cos n from 1-2sin^2(n/2), U = sqrt2*sin(n)/n * v). Pair
phase on VectorE: rotation by two cross products, quaternion product, log map
via arctan. Cross-product / quaternion-product partial products are emitted
as merged 6C-wide DVE ops (4D access patterns pairing two 3-vector products
per instruction). Unary work (squares, LUTs, |x| + accumulate via accum_out)
runs on ScalarE; activation-table switches are grouped (row Ln/Exp+Sin of
tile 0 first so VectorE starts early, then tiles 1-2, then per-tile pair
Ln/Exp+Arctan). The translation-delta g1 for tiles 0-1 is computed early to
fill VectorE's wait on the scalar LUT prologue. Final mean-L1 partial sums
(6 x [128,1] f32) DMA straight to HBM; the host does the last reduction.
"""
import os
import numpy as np
import ml_dtypes

N_CORES = 8
T_FULL = 2_000_000
NPAIRS = T_FULL - 1          # 1_999_999
D = 1956                     # pairs per partition per core
C = 652                      # pairs per tile (3 tiles)
NT = 3
R = C + 1
PPC = 128 * D                # 250_368 pairs per core
PAIRS_PAD = N_CORES * PPC    # 2_002_944
ROWS_PAD = PAIRS_PAD + 1

PRED_LEN = 6 * (PPC + 1)
TARG_LEN = 6 * PPC

LN2 = float(np.log(2.0))
LN2SQ2 = float(np.log(2.0 * np.sqrt(2.0)))   # i2n carries 2*sqrt2
PI2 = float(np.pi / 2.0)
SQ2 = float(np.sqrt(2.0))

_BUILT = {}


def _patch_act_tables():
    import concourse.bacc as bacc_mod
    import concourse.hw_specs as hw

    if getattr(bacc_mod, "_vo_tables_patched", False):
        return
    orig = hw.get_activation_tables

    def steered(arch, _orig=orig):
        from concourse import mybir as _mb
        AF = _mb.ActivationFunctionType
        t = {k: set(v) for k, v in _orig(arch).items()}
        # Keep all 24 entries (act_func_set_id indexes the original list);
        # drop ln/exp/arctan from the earlier sets so the table-load pass
        # resolves them to natural_log_exp_and_others / trig_and_small.
        t.get("natural_log", set()).discard(AF.Ln)
        t.get("exp_and_others", set()).discard(AF.Exp)
        t.get("sigmoid_and_others", set()).discard(AF.Arctan)
        return t

    bacc_mod.get_activation_tables = steered
    bacc_mod._vo_tables_patched = True


def _build():
    from concourse import bacc, tile, mybir
    from concourse.ap import AP
    from concourse.bass import _add_dep_helper

    _patch_act_tables()

    f32, bf16 = mybir.dt.float32, mybir.dt.bfloat16
    OP = mybir.AluOpType
    AF = mybir.ActivationFunctionType

    nc = bacc.Bacc("TRN2", target_bir_lowering=False, debug=False,
                   num_devices=N_CORES)
    pred_h = nc.declare_dram_parameter("pred", [PRED_LEN], bf16, isOutput=False)
    targ_h = nc.declare_dram_parameter("targ", [TARG_LEN], bf16, isOutput=False)
    out_h = nc.declare_dram_parameter("out", [128, 6], f32, isOutput=True)

    for v in (1e-16, -LN2, LN2SQ2, PI2):
        v = float(v)
        if (f32, v) not in nc.const_aps.aps:
            t = nc.alloc_sbuf_tensor(f"uconst-{v}", [128, 1], f32)
            nc.gpsimd.memset(t.ap(), v)
            nc.const_aps.aps[(f32, v)] = t.ap()
    nc.all_engine_barrier()

    PL_P = PPC + 1   # pred plane length
    PL_T = PPC       # targ plane length

    def sb(tile_, off, dims):
        base = tile_[:, :]
        return AP(base.tensor, base.offset + off,
                  [[base.ap.to_list()[0][0], 128]] + dims)

    acc_ts, acc_qs = [], []
    groups = {}  # (tile, name) -> list of act instructions

    with tile.TileContext(nc) as tc:

        def mkact(tile_i, group, *args, **kw):
            ins = nc.scalar.activation(*args, **kw)
            if group is not None:
                groups.setdefault((tile_i, group), []).append(ins)
            return ins

        with (
            tc.tile_pool(name="inp", bufs=1) as pin,
            tc.tile_pool(name="rowp", bufs=1) as prow,
            tc.tile_pool(name="scr", bufs=1) as pscr,
            tc.tile_pool(name="accp", bufs=8) as pacc,
        ):
            state = {}

            def dma_phase(t):
                # ---- DMA: component-major bf16 planes, all contiguous ----
                tv = pin.tile([128, 3 * R], bf16, tag=f"tv{t}")   # logq comps
                nc.sync.dma_start(
                    tv[:].rearrange("p (c r) -> p c r", c=3),
                    AP(pred_h, 3 * PL_P + t * C, [[D, 128], [PL_P, 3], [1, R]]))
                tt = pin.tile([128, 3 * R], bf16, tag=f"tt{t}")   # t comps
                nc.sync.dma_start(
                    tt[:].rearrange("p (c r) -> p c r", c=3),
                    AP(pred_h, t * C, [[D, 128], [PL_P, 3], [1, R]]))
                gtt = pin.tile([128, 3 * C], bf16, tag=f"gtt{t}")  # targ t
                nc.sync.dma_start(
                    gtt[:].rearrange("p (c r) -> p c r", c=3),
                    AP(targ_h, t * C, [[D, 128], [PL_T, 3], [1, C]]))
                gtq = pin.tile([128, 3 * C], bf16, tag=f"gtq{t}")  # targ q
                nc.sync.dma_start(
                    gtq[:].rearrange("p (c r) -> p c r", c=3),
                    AP(targ_h, 3 * PL_T + t * C, [[D, 128], [PL_T, 3], [1, C]]))
                state[t] = {"tv": tv, "tt": tt, "gtt": gtt, "gtq": gtq}

            def row_n2(t):
                st = state[t]
                sq = pscr.tile([128, 3 * R], bf16, tag="sq")
                nc.vector.tensor_tensor(sq[:], st["tv"][:], st["tv"][:],
                                        OP.mult)
                n2a = pscr.tile([128, R], bf16, tag="n2a")
                nc.vector.tensor_tensor(n2a[:], sq[:, 0:R], sq[:, R:2 * R], OP.add)
                n2 = pscr.tile([128, R], bf16, tag=f"n2{t}")
                nc.vector.tensor_tensor(n2[:], n2a[:], sq[:, 2 * R:3 * R], OP.add)
                st["n2"] = n2

            def early_g1(t):
                # translation deltas: independent of the row LUT chain, runs
                # on VectorE while ScalarE does table loads + Ln/Exp/Sin.
                st = state[t]
                tt = st["tt"]
                g1 = pscr.tile([128, 5 * C], bf16, tag=f"g1{t}")
                nc.vector.tensor_tensor(
                    sb(g1, 0, [[C, 3], [1, C]]),
                    sb(tt, 1, [[R, 3], [1, C]]),
                    sb(tt, 0, [[R, 3], [1, C]]), OP.subtract)
                nc.vector.tensor_copy(g1[:, 3 * C:5 * C], g1[:, 0:2 * C])
                st["g1"] = g1

            def row_lut(t):
                st = state[t]
                l = pscr.tile([128, R], f32, tag="l")
                mkact(t, 'rowLE', l[:], st["n2"][:], AF.Ln, bias=1e-16)
                n4 = pscr.tile([128, R], f32, tag=f"n4{t}")
                mkact(t, 'rowLE', n4[:], l[:], AF.Exp, bias=-LN2, scale=0.5)
                i2n = pscr.tile([128, R], bf16, tag=f"i2n{t}")
                mkact(t, 'rowLE', i2n[:], l[:], AF.Exp, bias=LN2SQ2, scale=-0.5)
                st["n4"], st["i2n"] = n4, i2n

            def row_trig(t):
                st = state[t]
                s4 = pscr.tile([128, R], bf16, tag=f"s4{t}")
                mkact(t, 'rowTR', s4[:], st["n4"][:], AF.Sin)
                c4 = pscr.tile([128, R], bf16, tag=f"c4{t}")
                mkact(t, 'rowTR', c4[:], st["n4"][:], AF.Sin, bias=PI2)
                st["s4"], st["c4"] = s4, c4

            def row_fin(t):
                st = state[t]
                sc = pscr.tile([128, R], bf16, tag="scx")
                nc.vector.tensor_tensor(sc[:], st["s4"][:], st["c4"][:], OP.mult)
                s4sq = pscr.tile([128, R], bf16, tag="s4sq")
                mkact(t, None, s4sq[:], st["s4"][:], AF.Square)
                A = prow.tile([128, R], bf16, tag=f"A{t}")
                mkact(t, None, A[:], s4sq[:], AF.Copy, bias=SQ2, scale=-2.0 * SQ2)
                sn = pscr.tile([128, R], bf16, tag="sn")
                nc.vector.tensor_tensor(sn[:], sc[:], st["i2n"][:], OP.mult)
                U = prow.tile([128, 5 * R], bf16, tag=f"U{t}")
                nc.vector.tensor_tensor(
                    sb(U, 0, [[R, 3], [1, R]]),
                    st["tv"][:].rearrange("p (c r) -> p c r", c=3),
                    sb(sn, 0, [[0, 3], [1, R]]), OP.mult)
                nc.vector.tensor_copy(U[:, 3 * R:5 * R], U[:, 0:2 * R])
                st["A"], st["U"] = A, U

            def pair_helpers(t):
                st = state[t]
                A, U, tt = st["A"], st["U"], st["tt"]

                def A_at(row_off):
                    return sb(A, row_off, [[0, 3], [1, C]])

                def U_at(comp_rot, row_off):
                    return sb(U, comp_rot * R + row_off, [[R, 3], [1, C]])

                def TT_at(row_off):
                    return sb(tt, row_off, [[R, 3], [1, C]])

                cmC = lambda tl: sb(tl, 0, [[C, 3], [1, C]])
                return st, A, U, A_at, U_at, TT_at, cmC

            def pair_t_branch(t):
                st, A, U, A_at, U_at, TT_at, cmC = pair_helpers(t)
                # ----- translation part -----
                if "g1" in st:
                    g1 = st["g1"]
                else:
                    g1 = pscr.tile([128, 5 * C], bf16, tag="g10")
                    nc.vector.tensor_tensor(cmC(g1), TT_at(1), TT_at(0),
                                            OP.subtract)
                    nc.vector.tensor_copy(g1[:, 3 * C:5 * C], g1[:, 0:2 * C])

                def G1(comp_rot):
                    return sb(g1, comp_rot * C, [[C, 3], [1, C]])

                pp = pscr.tile([128, 6 * C], bf16, tag="pp")
                nc.vector.tensor_tensor(
                    sb(pp, 0, [[3 * C, 2], [C, 3], [1, C]]),
                    sb(U, R, [[R, 2], [R, 3], [1, C]]),
                    sb(g1, 2 * C, [[-C, 2], [C, 3], [1, C]]), OP.mult)
                b = pscr.tile([128, 5 * C], bf16, tag="b")
                nc.vector.tensor_tensor(cmC(b), pp[:, 0:3 * C],
                                        pp[:, 3 * C:6 * C], OP.subtract)
                nc.vector.tensor_copy(b[:, 3 * C:5 * C], b[:, 0:2 * C])

                def B(comp_rot):
                    return sb(b, comp_rot * C, [[C, 3], [1, C]])

                qq = pscr.tile([128, 6 * C], bf16, tag="pp")
                nc.vector.tensor_tensor(
                    sb(qq, 0, [[3 * C, 2], [C, 3], [1, C]]),
                    sb(U, R, [[R, 2], [R, 3], [1, C]]),
                    sb(b, 2 * C, [[-C, 2], [C, 3], [1, C]]), OP.mult)
                cp = pscr.tile([128, 3 * C], bf16, tag="cp")
                nc.vector.tensor_tensor(cmC(cp), qq[:, 0:3 * C],
                                        qq[:, 3 * C:6 * C], OP.subtract)
                m = pscr.tile([128, 3 * C], bf16, tag="m")
                nc.vector.tensor_tensor(cmC(m), A_at(0), B(0), OP.mult)

                g = pscr.tile([128, 3 * C], bf16, tag="gg")
                nc.vector.tensor_tensor(g[:], g1[:, 0:3 * C], st["gtt"][:],
                                        OP.subtract)
                gc = pscr.tile([128, 3 * C], bf16, tag="gc")
                nc.vector.tensor_tensor(gc[:], g[:], cp[:], OP.add)
                dfft = pscr.tile([128, 3 * C], bf16, tag="dfft")
                nc.vector.tensor_tensor(dfft[:], gc[:], m[:], OP.subtract)
                st["dfft"] = dfft

            def pair_abs_t(t):
                st = state[t]
                dump_t = pscr.tile([128, 3 * R], bf16, tag="sq")
                acc_t = pacc.tile([128, 1], f32, tag="acct")
                mkact(t, None, dump_t[:, 0:3 * C], st["dfft"][:], AF.Abs,
                      accum_out=acc_t[:])
                acc_ts.append(acc_t)

            def pair_q_branch(t, sq_on_v=False, abs_t_after_sq=False):
                st, A, U, A_at, U_at, TT_at, cmC = pair_helpers(t)
                # ----- rotation part: qV = A0*U1 - A1*U0 - U0 x U1 (= 2*qv) --
                mA = pscr.tile([128, C], bf16, tag="mA")
                nc.vector.tensor_tensor(mA[:], A[:, 0:C], A[:, 1:1 + C], OP.mult)
                mc = pscr.tile([128, 6 * C], bf16, tag="pp")
                nc.vector.tensor_tensor(
                    sb(mc, 0, [[3 * C, 2], [C, 3], [1, C]]),
                    sb(U, 0, [[R, 2], [R, 3], [1, C]]),
                    sb(U, 1, [[2 * R, 2], [R, 3], [1, C]]), OP.mult)
                s1 = pscr.tile([128, C], bf16, tag="s1")
                nc.vector.tensor_tensor(s1[:], mc[:, 0:C], mc[:, C:2 * C], OP.add)
                s2 = pscr.tile([128, C], bf16, tag="s2")
                nc.vector.tensor_tensor(s2[:], s1[:], mc[:, 2 * C:3 * C], OP.add)
                qs2 = pscr.tile([128, C], bf16, tag="qs2")
                nc.vector.tensor_tensor(qs2[:], s2[:], mA[:], OP.add)

                c2 = pscr.tile([128, 3 * C], bf16, tag="p2")
                nc.vector.tensor_tensor(cmC(c2), U_at(2, 0), U_at(1, 1), OP.mult)
                cr = pscr.tile([128, 3 * C], bf16, tag="cr")
                nc.vector.tensor_tensor(cr[:], mc[:, 3 * C:6 * C], c2[:],
                                        OP.subtract)
                pq = pscr.tile([128, 6 * C], bf16, tag="pp")
                nc.vector.tensor_tensor(
                    sb(pq, 0, [[3 * C, 2], [C, 3], [1, C]]),
                    sb(A, 0, [[1, 2], [0, 3], [1, C]]),
                    sb(U, 1, [[-1, 2], [R, 3], [1, C]]), OP.mult)
                w1 = pscr.tile([128, 3 * C], bf16, tag="w1")
                nc.vector.tensor_tensor(w1[:], pq[:, 0:3 * C],
                                        pq[:, 3 * C:6 * C], OP.subtract)
                qV = pscr.tile([128, 3 * C], bf16, tag=f"qV{t}")
                nc.vector.tensor_tensor(qV[:], w1[:], cr[:], OP.subtract)

                qVsq = pscr.tile([128, 3 * C], bf16, tag="p1")
                if sq_on_v:
                    nc.vector.tensor_tensor(qVsq[:], qV[:], qV[:], OP.mult)
                else:
                    mkact(t, None, qVsq[:], qV[:], AF.Square)
                if abs_t_after_sq:
                    pair_abs_t(t)
                nva = pscr.tile([128, C], bf16, tag="s1")
                nc.vector.tensor_tensor(nva[:], qVsq[:, 0:C], qVsq[:, C:2 * C],
                                        OP.add)
                nv2 = pscr.tile([128, C], bf16, tag="s2")
                nc.vector.tensor_tensor(nv2[:], nva[:], qVsq[:, 2 * C:3 * C],
                                        OP.add)

                lq = pscr.tile([128, C], f32, tag="lq")
                mkact(t, 'pairLE', lq[:], nv2[:], AF.Ln, bias=1e-16)
                rs = pscr.tile([128, C], bf16, tag=f"rs{t}")
                mkact(t, 'pairLE', rs[:], lq[:], AF.Exp, scale=-0.5)
                r2 = pscr.tile([128, C], bf16, tag=f"r2{t}")
                nc.vector.tensor_tensor(r2[:], qs2[:], rs[:], OP.mult)
                st["qV"], st["rs"], st["r2"] = qV, rs, r2

            def pair_at(t):
                st = state[t]
                at = pscr.tile([128, C], f32, tag="at")
                mkact(t, 'pairTR', at[:], st["r2"][:], AF.Arctan, scale=-1.0)
                st["at"] = at
                if t < 2:
                    atp = pscr.tile([128, C], bf16, tag="atp")
                    mkact(t, None, atp[:], at[:], AF.Copy, bias=PI2)
                    st["atp"] = atp

            def pair_b_rest(t):
                st = state.pop(t)
                ratio = pscr.tile([128, C], bf16, tag="ratio")
                if "atp" in st:
                    nc.vector.tensor_tensor(ratio[:], st["atp"][:], st["rs"][:],
                                            OP.mult)
                else:
                    nc.vector.scalar_tensor_tensor(ratio[:], st["at"][:], PI2,
                                                   st["rs"][:], OP.add, OP.mult)
                ld = pscr.tile([128, 3 * C], bf16, tag="w1")
                nc.vector.tensor_tensor(
                    sb(ld, 0, [[C, 3], [1, C]]),
                    sb(st["qV"], 0, [[C, 3], [1, C]]),
                    sb(ratio, 0, [[0, 3], [1, C]]), OP.mult)
                ldiff = pscr.tile([128, 3 * C], bf16, tag="cr")
                nc.vector.tensor_tensor(ldiff[:], ld[:], st["gtq"][:], OP.subtract)
                dump_q = pscr.tile([128, 3 * R], bf16, tag="sq")
                acc_q = pacc.tile([128, 1], f32, tag="accq")
                mkact(t, None, dump_q[:, 0:3 * C], ldiff[:], AF.Abs,
                      accum_out=acc_q[:])
                acc_qs.append(acc_q)

            for t in range(NT):
                dma_phase(t)
            row_n2(0)
            row_lut(0)
            row_trig(0)
            row_n2(1)
            row_n2(2)
            early_g1(0)
            early_g1(1)
            row_lut(1)
            row_lut(2)
            row_trig(1)
            row_trig(2)
            for t in range(NT):
                row_fin(t)
            for t in range(NT):
                pair_t_branch(t)
                pair_q_branch(t, abs_t_after_sq=True, sq_on_v=(t == 2))
                pair_at(t)
                pair_b_rest(t)

            # Chain LUT activations so same-table-set groups run contiguously
            # across tiles: 4 table loads total (rowLE, rowTR, pairLE, pairTR).
            order = [('rowLE', 0), ('rowTR', 0),
                     ('rowLE', 1), ('rowLE', 2),
                     ('rowTR', 1), ('rowTR', 2)]
            for ti in range(NT):
                order.append(('pairLE', ti))
                order.append(('pairTR', ti))
            seq = []
            for gname, ti in order:
                seq.extend(groups.get((ti, gname), []))
            for i in range(1, len(seq)):
                _add_dep_helper(seq[i].ins, seq[i - 1].ins, False,
                                "act table-set grouping")

            for i, acc in enumerate(acc_ts + acc_qs):
                nc.sync.dma_start(out_h[:, i:i + 1], acc[:])

    nc.compile()
    return nc


def _get_nc():
    if "nc" not in _BUILT:
        _BUILT["nc"] = _build()
    return _BUILT["nc"]


def run_device(pred, targ, trace=False):
    """pred: (1,T,6) f32, targ: (1,T-1,6) f32 -> (sum|dt|, sum|dq|, exec_ns)"""
    from concourse.bass_utils import run_bass_kernel_spmd

    nc = _get_nc()
    p = np.asarray(pred, dtype=np.float32).reshape(-1, 6)
    g = np.asarray(targ, dtype=np.float32).reshape(-1, 6)
    n_dup = ROWS_PAD - p.shape[0]
    p_pad = np.concatenate([p, np.repeat(p[-1:], n_dup, axis=0)], axis=0)
    g_pad = np.concatenate(
        [g, np.zeros((PAIRS_PAD - g.shape[0], 6), np.float32)], axis=0)

    in_maps = []
    for c in range(N_CORES):
        s = c * PPC
        in_maps.append({
            "pred": np.ascontiguousarray(p_pad[s:s + PPC + 1].T)
                     .astype(ml_dtypes.bfloat16).reshape(-1),
            "targ": np.ascontiguousarray(g_pad[s:s + PPC].T)
                     .astype(ml_dtypes.bfloat16).reshape(-1),
        })
    res = run_bass_kernel_spmd(nc, in_maps, core_ids=list(range(N_CORES)),
                               trace=trace)
    psum = np.stack([res.results[i]["out"] for i in range(N_CORES)])
    st = float(psum[:, :, 0:3].sum(dtype=np.float64))
    sq = float(psum[:, :, 3:6].sum(dtype=np.float64))
    return st, sq, res.exec_time_ns


def kernel(pred, targ, srx, srq):
    trace = bool(int(os.environ.get("VO_KERNEL_TRACE", "0")))
    st, sq, _ = run_device(pred, targ, trace=trace)
    t_loss = st / (3.0 * NPAIRS)
    q_loss = sq / (3.0 * NPAIRS)
    srx_v = float(np.asarray(srx).reshape(-1)[0])
    srq_v = float(np.asarray(srq).reshape(-1)[0])
    out = (np.exp(-srx_v) * t_loss + srx_v +
           np.exp(-srq_v) * q_loss + srq_v)
    return np.array([out], dtype=np.float32)


# revision 36
# speedup vs baseline: 1.1755x; 1.0081x over previous
"""Trainium2 Bass kernel for nn_AtLocPlusCriterion_VO.

loss = exp(-srx)*mean|vo_t - tg_t| + srx + exp(-srq)*mean|vo_q - tg_q| + srq
with vo = calc_vo_logq(pred[:-1], pred[1:]) (relative SE(3) pose, log-quaternion).

Sequence-parallel across 8 NeuronCores (1-row halo per shard). Inputs are
resharded host-side into component-major (SoA) bf16 planes so every on-device
vector op runs in the DVE 2x bf16 mode. Per core: 1956 pairs per SBUF
partition, 3 tiles of 652.

Row phase: n^2 on VectorE, 1/n and n/2 via Ln/Exp, half-angle sin/cos via the
Sin LUT (A = sqrt2*cos n from 1-2sin^2(n/2), U = sqrt2*sin(n)/n * v). Pair
phase on VectorE: rotation by two cross products, quaternion product, log map
via arctan. Cross-product / quaternion-product partial products are emitted
as merged 6C-wide DVE ops (4D access patterns pairing two 3-vector products
per instruction). Unary work (squares, LUTs, |x| + accumulate via accum_out)
runs on ScalarE; activation-table switches are grouped (row Ln/Exp+Sin of
tile 0 first so VectorE starts early, then tiles 1-2, then per-tile pair
Ln/Exp+Arctan). The translation-delta g1 for tiles 0-1 is computed early to
fill VectorE's wait on the scalar LUT prologue. Final mean-L1 partial sums
(6 x [128,1] f32) DMA straight to HBM; the host does the last reduction.
"""
import os
import numpy as np
import ml_dtypes

N_CORES = 8
T_FULL = 2_000_000
NPAIRS = T_FULL - 1          # 1_999_999
D = 1956                     # pairs per partition per core
C = 652                      # pairs per tile (3 tiles)
NT = 3
R = C + 1
PPC = 128 * D                # 250_368 pairs per core
PAIRS_PAD = N_CORES * PPC    # 2_002_944
ROWS_PAD = PAIRS_PAD + 1

PRED_LEN = 6 * (PPC + 1)
TARG_LEN = 6 * PPC

LN2 = float(np.log(2.0))
LN2SQ2 = float(np.log(2.0 * np.sqrt(2.0)))   # i2n carries 2*sqrt2
PI2 = float(np.pi / 2.0)
SQ2 = float(np.sqrt(2.0))

_BUILT = {}


def _patch_act_tables():
    import concourse.bacc as bacc_mod
    import concourse.hw_specs as hw

    if getattr(bacc_mod, "_vo_tables_patched", False):
        return
    orig = hw.get_activation_tables

    def steered(arch, _orig=orig):
        from concourse import mybir as _mb
        AF = _mb.ActivationFunctionType
        t = {k: set(v) for k, v in _orig(arch).items()}
        # Keep all 24 entries (act_func_set_id indexes the original list);
        # drop ln/exp/arctan from the earlier sets so the table-load pass
        # resolves them to natural_log_exp_and_others / trig_and_small.
        t.get("natural_log", set()).discard(AF.Ln)
        t.get("exp_and_others", set()).discard(AF.Exp)
        t.get("sigmoid_and_others", set()).discard(AF.Arctan)
        return t

    bacc_mod.get_activation_tables = steered
    bacc_mod._vo_tables_patched = True


def _build():
    from concourse import bacc, tile, mybir
    from concourse.ap import AP
    from concourse.bass import _add_dep_helper

    _patch_act_tables()

    f32, bf16 = mybir.dt.float32, mybir.dt.bfloat16
    OP = mybir.AluOpType
    AF = mybir.ActivationFunctionType

    nc = bacc.Bacc("TRN2", target_bir_lowering=False, debug=False,
                   num_devices=N_CORES)
    pred_h = nc.declare_dram_parameter("pred", [PRED_LEN], bf16, isOutput=False)
    targ_h = nc.declare_dram_parameter("targ", [TARG_LEN], bf16, isOutput=False)
    out_h = nc.declare_dram_parameter("out", [128, 6], f32, isOutput=True)

    for v in (1e-16, -LN2, LN2SQ2, PI2):
        v = float(v)
        if (f32, v) not in nc.const_aps.aps:
            t = nc.alloc_sbuf_tensor(f"uconst-{v}", [128, 1], f32)
            nc.gpsimd.memset(t.ap(), v)
            nc.const_aps.aps[(f32, v)] = t.ap()
    nc.all_engine_barrier()

    PL_P = PPC + 1   # pred plane length
    PL_T = PPC       # targ plane length

    def sb(tile_, off, dims):
        base = tile_[:, :]
        return AP(base.tensor, base.offset + off,
                  [[base.ap.to_list()[0][0], 128]] + dims)

    acc_ts, acc_qs = [], []
    groups = {}  # (tile, name) -> list of act instructions

    with tile.TileContext(nc) as tc:

        def mkact(tile_i, group, *args, **kw):
            ins = nc.scalar.activation(*args, **kw)
            if group is not None:
                groups.setdefault((tile_i, group), []).append(ins)
            return ins

        with (
            tc.tile_pool(name="inp", bufs=1) as pin,
            tc.tile_pool(name="rowp", bufs=1) as prow,
            tc.tile_pool(name="scr", bufs=1) as pscr,
            tc.tile_pool(name="accp", bufs=8) as pacc,
        ):
            state = {}

            def dma_phase(t):
                # ---- DMA: component-major bf16 planes, all contiguous ----
                tv = pin.tile([128, 3 * R], bf16, tag=f"tv{t}")   # logq comps
                nc.sync.dma_start(
                    tv[:].rearrange("p (c r) -> p c r", c=3),
                    AP(pred_h, 3 * PL_P + t * C, [[D, 128], [PL_P, 3], [1, R]]))
                tt = pin.tile([128, 3 * R], bf16, tag=f"tt{t}")   # t comps
                nc.sync.dma_start(
                    tt[:].rearrange("p (c r) -> p c r", c=3),
                    AP(pred_h, t * C, [[D, 128], [PL_P, 3], [1, R]]))
                gtt = pin.tile([128, 3 * C], bf16, tag=f"gtt{t}")  # targ t
                nc.sync.dma_start(
                    gtt[:].rearrange("p (c r) -> p c r", c=3),
                    AP(targ_h, t * C, [[D, 128], [PL_T, 3], [1, C]]))
                gtq = pin.tile([128, 3 * C], bf16, tag=f"gtq{t}")  # targ q
                nc.sync.dma_start(
                    gtq[:].rearrange("p (c r) -> p c r", c=3),
                    AP(targ_h, 3 * PL_T + t * C, [[D, 128], [PL_T, 3], [1, C]]))
                state[t] = {"tv": tv, "tt": tt, "gtt": gtt, "gtq": gtq}

            def row_n2(t):
                st = state[t]
                sq = pscr.tile([128, 3 * R], bf16, tag="sq")
                nc.vector.tensor_tensor(sq[:], st["tv"][:], st["tv"][:],
                                        OP.mult)
                n2a = pscr.tile([128, R], bf16, tag="n2a")
                nc.vector.tensor_tensor(n2a[:], sq[:, 0:R], sq[:, R:2 * R], OP.add)
                n2 = pscr.tile([128, R], bf16, tag=f"n2{t}")
                nc.vector.tensor_tensor(n2[:], n2a[:], sq[:, 2 * R:3 * R], OP.add)
                st["n2"] = n2

            def early_g1(t):
                # translation deltas: independent of the row LUT chain, runs
                # on VectorE while ScalarE does table loads + Ln/Exp/Sin.
                st = state[t]
                tt = st["tt"]
                g1 = pscr.tile([128, 5 * C], bf16, tag=f"g1{t}")
                nc.vector.tensor_tensor(
                    sb(g1, 0, [[C, 3], [1, C]]),
                    sb(tt, 1, [[R, 3], [1, C]]),
                    sb(tt, 0, [[R, 3], [1, C]]), OP.subtract)
                nc.vector.tensor_copy(g1[:, 3 * C:5 * C], g1[:, 0:2 * C])
                st["g1"] = g1

            def row_lut(t):
                st = state[t]
                l = pscr.tile([128, R], f32, tag="l")
                mkact(t, 'rowLE', l[:], st["n2"][:], AF.Ln, bias=1e-16)
                n4 = pscr.tile([128, R], f32, tag=f"n4{t}")
                mkact(t, 'rowLE', n4[:], l[:], AF.Exp, bias=-LN2, scale=0.5)
                i2n = pscr.tile([128, R], bf16, tag=f"i2n{t}")
                mkact(t, 'rowLE', i2n[:], l[:], AF.Exp, bias=LN2SQ2, scale=-0.5)
                st["n4"], st["i2n"] = n4, i2n

            def row_trig(t):
                st = state[t]
                s4 = pscr.tile([128, R], bf16, tag=f"s4{t}")
                mkact(t, 'rowTR', s4[:], st["n4"][:], AF.Sin)
                c4 = pscr.tile([128, R], bf16, tag=f"c4{t}")
                mkact(t, 'rowTR', c4[:], st["n4"][:], AF.Sin, bias=PI2)
                st["s4"], st["c4"] = s4, c4

            def row_fin(t):
                st = state[t]
                sc = pscr.tile([128, R], bf16, tag="scx")
                nc.vector.tensor_tensor(sc[:], st["s4"][:], st["c4"][:], OP.mult)
                s4sq = pscr.tile([128, R], bf16, tag="s4sq")
                mkact(t, None, s4sq[:], st["s4"][:], AF.Square)
                A = prow.tile([128, R], bf16, tag=f"A{t}")
                mkact(t, None, A[:], s4sq[:], AF.Copy, bias=SQ2, scale=-2.0 * SQ2)
                sn = pscr.tile([128, R], bf16, tag="sn")
                nc.vector.tensor_tensor(sn[:], sc[:], st["i2n"][:], OP.mult)
                U = prow.tile([128, 5 * R], bf16, tag=f"U{t}")
                nc.vector.tensor_tensor(
                    sb(U, 0, [[R, 3], [1, R]]),
                    st["tv"][:].rearrange("p (c r) -> p c r", c=3),
                    sb(sn, 0, [[0, 3], [1, R]]), OP.mult)
                nc.vector.tensor_copy(U[:, 3 * R:5 * R], U[:, 0:2 * R])
                st["A"], st["U"] = A, U

            def pair_helpers(t):
                st = state[t]
                A, U, tt = st["A"], st["U"], st["tt"]

                def A_at(row_off):
                    return sb(A, row_off, [[0, 3], [1, C]])

                def U_at(comp_rot, row_off):
                    return sb(U, comp_rot * R + row_off, [[R, 3], [1, C]])

                def TT_at(row_off):
                    return sb(tt, row_off, [[R, 3], [1, C]])

                cmC = lambda tl: sb(tl, 0, [[C, 3], [1, C]])
                return st, A, U, A_at, U_at, TT_at, cmC

            def pair_t_branch(t):
                st, A, U, A_at, U_at, TT_at, cmC = pair_helpers(t)
                # ----- translation part -----
                if "g1" in st:
                    g1 = st["g1"]
                else:
                    g1 = pscr.tile([128, 5 * C], bf16, tag="g10")
                    nc.vector.tensor_tensor(cmC(g1), TT_at(1), TT_at(0),
                                            OP.subtract)
                    nc.vector.tensor_copy(g1[:, 3 * C:5 * C], g1[:, 0:2 * C])

                def G1(comp_rot):
                    return sb(g1, comp_rot * C, [[C, 3], [1, C]])

                pp = pscr.tile([128, 6 * C], bf16, tag="pp")
                nc.vector.tensor_tensor(
                    sb(pp, 0, [[3 * C, 2], [C, 3], [1, C]]),
                    sb(U, R, [[R, 2], [R, 3], [1, C]]),
                    sb(g1, 2 * C, [[-C, 2], [C, 3], [1, C]]), OP.mult)
                b = pscr.tile([128, 5 * C], bf16, tag="b")
                nc.vector.tensor_tensor(cmC(b), pp[:, 0:3 * C],
                                        pp[:, 3 * C:6 * C], OP.subtract)
                nc.vector.tensor_copy(b[:, 3 * C:5 * C], b[:, 0:2 * C])

                def B(comp_rot):
                    return sb(b, comp_rot * C, [[C, 3], [1, C]])

                qq = pscr.tile([128, 6 * C], bf16, tag="pp")
                nc.vector.tensor_tensor(
                    sb(qq, 0, [[3 * C, 2], [C, 3], [1, C]]),
                    sb(U, R, [[R, 2], [R, 3], [1, C]]),
                    sb(b, 2 * C, [[-C, 2], [C, 3], [1, C]]), OP.mult)
                cp = pscr.tile([128, 3 * C], bf16, tag="cp")
                nc.vector.tensor_tensor(cmC(cp), qq[:, 0:3 * C],
                                        qq[:, 3 * C:6 * C], OP.subtract)
                m = pscr.tile([128, 3 * C], bf16, tag="m")
                nc.vector.tensor_tensor(cmC(m), A_at(0), B(0), OP.mult)

                g = pscr.tile([128, 3 * C], bf16, tag="gg")
                nc.vector.tensor_tensor(g[:], g1[:, 0:3 * C], st["gtt"][:],
                                        OP.subtract)
                gc = pscr.tile([128, 3 * C], bf16, tag="gc")
                nc.vector.tensor_tensor(gc[:], g[:], cp[:], OP.add)
                dfft = pscr.tile([128, 3 * C], bf16, tag="dfft")
                nc.vector.tensor_tensor(dfft[:], gc[:], m[:], OP.subtract)
                st["dfft"] = dfft

            def pair_abs_t(t):
                st = state[t]
                dump_t = pscr.tile([128, 3 * R], bf16, tag="sq")
                acc_t = pacc.tile([128, 1], f32, tag="acct")
                mkact(t, None, dump_t[:, 0:3 * C], st["dfft"][:], AF.Abs,
                      accum_out=acc_t[:])
                acc_ts.append(acc_t)

            def pair_q_branch(t, sq_on_v=False, abs_t_after_sq=False):
                st, A, U, A_at, U_at, TT_at, cmC = pair_helpers(t)
                # ----- rotation part: qV = A0*U1 - A1*U0 - U0 x U1 (= 2*qv) --
                mA = pscr.tile([128, C], bf16, tag="mA")
                nc.vector.tensor_tensor(mA[:], A[:, 0:C], A[:, 1:1 + C], OP.mult)
                mc = pscr.tile([128, 6 * C], bf16, tag="pp")
                nc.vector.tensor_tensor(
                    sb(mc, 0, [[3 * C, 2], [C, 3], [1, C]]),
                    sb(U, 0, [[R, 2], [R, 3], [1, C]]),
                    sb(U, 1, [[2 * R, 2], [R, 3], [1, C]]), OP.mult)
                s1 = pscr.tile([128, C], bf16, tag="s1")
                nc.vector.tensor_tensor(s1[:], mc[:, 0:C], mc[:, C:2 * C], OP.add)
                s2 = pscr.tile([128, C], bf16, tag="s2")
                nc.vector.tensor_tensor(s2[:], s1[:], mc[:, 2 * C:3 * C], OP.add)
                qs2 = pscr.tile([128, C], bf16, tag="qs2")
                nc.vector.tensor_tensor(qs2[:], s2[:], mA[:], OP.add)

                c2 = pscr.tile([128, 3 * C], bf16, tag="p2")
                nc.vector.tensor_tensor(cmC(c2), U_at(2, 0), U_at(1, 1), OP.mult)
                cr = pscr.tile([128, 3 * C], bf16, tag="cr")
                nc.vector.tensor_tensor(cr[:], mc[:, 3 * C:6 * C], c2[:],
                                        OP.subtract)
                pq = pscr.tile([128, 6 * C], bf16, tag="pp")
                nc.vector.tensor_tensor(
                    sb(pq, 0, [[3 * C, 2], [C, 3], [1, C]]),
                    sb(A, 0, [[1, 2], [0, 3], [1, C]]),
                    sb(U, 1, [[-1, 2], [R, 3], [1, C]]), OP.mult)
                w1 = pscr.tile([128, 3 * C], bf16, tag="w1")
                nc.vector.tensor_tensor(w1[:], pq[:, 0:3 * C],
                                        pq[:, 3 * C:6 * C], OP.subtract)
                qV = pscr.tile([128, 3 * C], bf16, tag=f"qV{t}")
                nc.vector.tensor_tensor(qV[:], w1[:], cr[:], OP.subtract)

                qVsq = pscr.tile([128, 3 * C], bf16, tag="p1")
                if sq_on_v:
                    nc.vector.tensor_tensor(qVsq[:], qV[:], qV[:], OP.mult)
                else:
                    mkact(t, None, qVsq[:], qV[:], AF.Square)
                if abs_t_after_sq:
                    pair_abs_t(t)
                nva = pscr.tile([128, C], bf16, tag="s1")
                nc.vector.tensor_tensor(nva[:], qVsq[:, 0:C], qVsq[:, C:2 * C],
                                        OP.add)
                nv2 = pscr.tile([128, C], bf16, tag="s2")
                nc.vector.tensor_tensor(nv2[:], nva[:], qVsq[:, 2 * C:3 * C],
                                        OP.add)

                lq = pscr.tile([128, C], f32, tag="lq")
                mkact(t, 'pairLE', lq[:], nv2[:], AF.Ln, bias=1e-16)
                rs = pscr.tile([128, C], bf16, tag=f"rs{t}")
                mkact(t, 'pairLE', rs[:], lq[:], AF.Exp, scale=-0.5)
                r2 = pscr.tile([128, C], bf16, tag=f"r2{t}")
                nc.vector.tensor_tensor(r2[:], qs2[:], rs[:], OP.mult)
                st["qV"], st["rs"], st["r2"] = qV, rs, r2

            def pair_at(t):
                st = state[t]
                at = pscr.tile([128, C], f32, tag="at")
                mkact(t, 'pairTR', at[:], st["r2"][:], AF.Arctan, scale=-1.0)
                st["at"] = at
                if t < 2:
                    atp = pscr.tile([128, C], bf16, tag="atp")
                    mkact(t, None, atp[:], at[:], AF.Copy, bias=PI2)
                    st["atp"] = atp

            def pair_b_rest(t):
                st = state.pop(t)
                ratio = pscr.tile([128, C], bf16, tag="ratio")
                if "atp" in st:
                    nc.vector.tensor_tensor(ratio[:], st["atp"][:], st["rs"][:],
                                            OP.mult)
                else:
                    nc.vector.scalar_tensor_tensor(ratio[:], st["at"][:], PI2,
                                                   st["rs"][:], OP.add, OP.mult)
                ld = pscr.tile([128, 3 * C], bf16, tag="w1")
                nc.vector.tensor_tensor(
                    sb(ld, 0, [[C, 3], [1, C]]),
                    sb(st["qV"], 0, [[C, 3], [1, C]]),
                    sb(ratio, 0, [[0, 3], [1, C]]), OP.mult)
                ldiff = pscr.tile([128, 3 * C], bf16, tag="cr")
                nc.vector.tensor_tensor(ldiff[:], ld[:], st["gtq"][:], OP.subtract)
                acc_q = pacc.tile([128, 1], f32, tag="accq")
                if t == 2:
                    dump_q = pscr.tile([128, 3 * C], bf16, tag="dfft")
                    nc.vector.scalar_tensor_tensor(dump_q[:], ldiff[:], -1.0,
                                                   ldiff[:], OP.mult, OP.max,
                                                   accum_out=acc_q[:])
                else:
                    dump_q = pscr.tile([128, 3 * R], bf16, tag="sq")
                    mkact(t, None, dump_q[:, 0:3 * C], ldiff[:], AF.Abs,
                          accum_out=acc_q[:])
                acc_qs.append(acc_q)

            for t in range(NT):
                dma_phase(t)
            row_n2(0)
            row_lut(0)
            row_trig(0)
            row_n2(1)
            row_n2(2)
            early_g1(0)
            early_g1(1)
            row_lut(1)
            row_lut(2)
            row_trig(1)
            row_trig(2)
            for t in range(NT):
                row_fin(t)
            for t in range(NT):
                pair_t_branch(t)
                pair_q_branch(t, abs_t_after_sq=True, sq_on_v=(t == 2))
                pair_at(t)
                pair_b_rest(t)

            # Chain LUT activations so same-table-set groups run contiguously
            # across tiles: 4 table loads total (rowLE, rowTR, pairLE, pairTR).
            order = [('rowLE', 0), ('rowTR', 0),
                     ('rowLE', 1), ('rowLE', 2),
                     ('rowTR', 1), ('rowTR', 2)]
            for ti in range(NT):
                order.append(('pairLE', ti))
                order.append(('pairTR', ti))
            seq = []
            for gname, ti in order:
                seq.extend(groups.get((ti, gname), []))
            for i in range(1, len(seq)):
                _add_dep_helper(seq[i].ins, seq[i - 1].ins, False,
                                "act table-set grouping")

            for i, acc in enumerate(acc_ts + acc_qs):
                nc.sync.dma_start(out_h[:, i:i + 1], acc[:])

    nc.compile()
    return nc


def _get_nc():
    if "nc" not in _BUILT:
        _BUILT["nc"] = _build()
    return _BUILT["nc"]


def run_device(pred, targ, trace=False):
    """pred: (1,T,6) f32, targ: (1,T-1,6) f32 -> (sum|dt|, sum|dq|, exec_ns)"""
    from concourse.bass_utils import run_bass_kernel_spmd

    nc = _get_nc()
    p = np.asarray(pred, dtype=np.float32).reshape(-1, 6)
    g = np.asarray(targ, dtype=np.float32).reshape(-1, 6)
    n_dup = ROWS_PAD - p.shape[0]
    p_pad = np.concatenate([p, np.repeat(p[-1:], n_dup, axis=0)], axis=0)
    g_pad = np.concatenate(
        [g, np.zeros((PAIRS_PAD - g.shape[0], 6), np.float32)], axis=0)

    in_maps = []
    for c in range(N_CORES):
        s = c * PPC
        in_maps.append({
            "pred": np.ascontiguousarray(p_pad[s:s + PPC + 1].T)
                     .astype(ml_dtypes.bfloat16).reshape(-1),
            "targ": np.ascontiguousarray(g_pad[s:s + PPC].T)
                     .astype(ml_dtypes.bfloat16).reshape(-1),
        })
    res = run_bass_kernel_spmd(nc, in_maps, core_ids=list(range(N_CORES)),
                               trace=trace)
    psum = np.stack([res.results[i]["out"] for i in range(N_CORES)])
    st = float(psum[:, :, 0:3].sum(dtype=np.float64))
    sq = float(psum[:, :, 3:6].sum(dtype=np.float64))
    return st, sq, res.exec_time_ns


def kernel(pred, targ, srx, srq):
    trace = bool(int(os.environ.get("VO_KERNEL_TRACE", "0")))
    st, sq, _ = run_device(pred, targ, trace=trace)
    t_loss = st / (3.0 * NPAIRS)
    q_loss = sq / (3.0 * NPAIRS)
    srx_v = float(np.asarray(srx).reshape(-1)[0])
    srq_v = float(np.asarray(srq).reshape(-1)[0])
    out = (np.exp(-srx_v) * t_loss + srx_v +
           np.exp(-srq_v) * q_loss + srq_v)
    return np.array([out], dtype=np.float32)
